# revision 2
# baseline (speedup 1.0000x reference)
"""Trainium2 Bass kernel for BiDecoder edge dot products.

out[e] = dot(ufeat[src[e]], ifeat[dst[e]])   for E=300000 edges, D=256.

Strategy (8 NeuronCores, SPMD):
  - Shard edges across the 8 cores (37500 each); replicate ufeat, and give
    each core a host-compacted ifeat table holding only its distinct dst
    rows (~26.4k < 32767, so one int16 gather base — no 32768-row split).
  - Tables are cast to bf16 on host (rel err ~1e-3 << 2e-2 gate): halves
    the gather traffic and doubles DVE throughput.
  - Per core, edges are sorted by dst (compacted hv rows are then read in
    ascending order) and, within each gather chunk, by src (hu reads
    ascend per chunk). Large 4096-row dma_gather calls amortize the
    per-call SWDGE fixed overhead on the Pool engine; 4 SWDGE queues
    rotate per call so generation overlaps transfers.
  - DVE affine_mul_reduce fuses multiply + row-sum (f32 accumulate); one
    final DMA writes all dots out. Host reorders to original edge order.
"""

import sys

for _p in ("/opt/trn_rl_repo",):
    if _p not in sys.path:
        sys.path.append(_p)

import numpy as np
import ml_dtypes

BF16 = ml_dtypes.bfloat16

P = 128
D = 256
E = 300000
NCORES = 8
ECORE = E // NCORES
N_GENE = 20000
N_CELL = 50000
C_TILE = 32              # tiles (of 128 edges) per gather chunk
CHUNK_E = C_TILE * P     # 4096 edges per dma_gather call
COLS = CHUNK_E // 16     # idx columns per chunk in the wrapped layout
NSLOT = 3                # buffer slots (chunk c uses slot c % NSLOT)

_PROGRAM_CACHE: dict = {}


def _cdiv(a, b):
    return -(-a // b)


def _wrap_idx(idx_i16: np.ndarray, nchunk: int) -> np.ndarray:
    """[nchunk*CHUNK_E] int16 -> [128, nchunk*COLS] dma_gather idx layout.

    Within each chunk, index i lives at partition i%16, column i//16; the
    16-partition block is replicated 8x down the 128 partitions.
    """
    w = idx_i16.reshape(nchunk, COLS, 16).transpose(2, 0, 1).reshape(16, nchunk * COLS)
    return np.ascontiguousarray(np.tile(w, (8, 1)))


def _build_program(nchunk: int, vcap: int, n_gene: int = N_GENE):
    import concourse.bacc as bacc
    import concourse.mybir as mybir
    from concourse.library_config import mlp

    ntiles = nchunk * C_TILE
    totcols = nchunk * COLS

    nc = bacc.Bacc("TRN2", debug=False, num_swdge_queues=4,
                   dynamic_dma_scratch_size=65536)
    ufeat = nc.dram_tensor("ufeat", [n_gene, D], mybir.dt.bfloat16, kind="ExternalInput")
    vtab = nc.dram_tensor("vtab", [vcap, D], mybir.dt.bfloat16, kind="ExternalInput")
    sidx = nc.dram_tensor("sidx", [P, totcols], mybir.dt.int16, kind="ExternalInput")
    didx = nc.dram_tensor("didx", [P, totcols], mybir.dt.int16, kind="ExternalInput")
    y = nc.dram_tensor("y", [P, ntiles], mybir.dt.float32, kind="ExternalOutput")

    with (
        nc.sbuf_tensor("hu", [P, NSLOT, C_TILE, D], mybir.dt.bfloat16) as hu,
        nc.sbuf_tensor("hv", [P, NSLOT, C_TILE, D], mybir.dt.bfloat16) as hv,
        nc.sbuf_tensor("sidx_sb", [P, totcols], mybir.dt.int16) as sidx_sb,
        nc.sbuf_tensor("didx_sb", [P, totcols], mybir.dt.int16) as didx_sb,
        nc.sbuf_tensor("osb", [P, ntiles], mybir.dt.float32) as osb,
        nc.semaphore("io") as io,
        nc.semaphore("cons") as cons,
        nc.semaphore("io2") as io2,
        nc.Block() as block,
        __import__("contextlib").ExitStack() as _stk,
    ):
        gu = [_stk.enter_context(nc.semaphore(f"gu{i}")) for i in range(NSLOT)]
        gv = [_stk.enter_context(nc.semaphore(f"gv{i}")) for i in range(NSLOT)]

        @block.gpsimd
        def _(gp):
            gp.load_library(mlp)
            gp.wait_ge(io, 32)
            for c in range(nchunk):
                s = c % NSLOT
                if c >= NSLOT:
                    gp.wait_ge(cons, c - NSLOT + 1)
                cols = slice(c * COLS, (c + 1) * COLS)
                gp.dma_gather(
                    hu[:, s], ufeat[:, :], sidx_sb[:, cols], CHUNK_E, CHUNK_E, D,
                    queue_num=(2 * c) % 4, single_packet=False,
                ).then_inc(gu[s], 16)
                gp.dma_gather(
                    hv[:, s], vtab[:, :], didx_sb[:, cols], CHUNK_E, CHUNK_E, D,
                    queue_num=(2 * c + 1) % 4, single_packet=False,
                ).then_inc(gv[s], 16)
            for s in range(NSLOT):
                cnt = (nchunk - s + NSLOT - 1) // NSLOT
                if cnt:
                    gp.wait_ge(gu[s], 16 * cnt)
                    gp.wait_ge(gv[s], 16 * cnt)

        @block.vector
        def _(v):
            for c in range(nchunk):
                s = c % NSLOT
                k = c // NSLOT + 1
                v.wait_ge(gu[s], 16 * k)
                v.wait_ge(gv[s], 16 * k)
                for t in range(C_TILE):
                    col = c * C_TILE + t
                    inst = v.affine_mul_reduce(
                        out=hv[:, s, t, :],
                        accum_out=osb[:, col : col + 1],
                        in0=hu[:, s, t, :],
                        in1=hv[:, s, t, :],
                        scale=1.0,
                        bias=0.0,
                    )
                    if t == C_TILE - 1:
                        inst.then_inc(cons, 1)

        @block.sync
        def _(sy):
            sy.dma_start(sidx_sb[:], sidx[:]).then_inc(io, 16)
            sy.dma_start(didx_sb[:], didx[:]).then_inc(io, 16)
            sy.wait_ge(cons, nchunk)
            sy.dma_start(y[:, :], osb[:, :]).then_inc(io2, 16)
            sy.wait_ge(io2, 16)

    nc.compile()
    return nc


def _prep_core(s_j, d_loc, ids_j, nchunk):
    """Build one core's slot arrays: wrapped int16 idx tensors + edge ids."""
    nslot_e = nchunk * CHUNK_E
    n = len(s_j)
    sidx = np.zeros(nslot_e, np.int16)
    didx = np.zeros(nslot_e, np.int16)
    eid = np.full(nslot_e, -1, np.int64)
    sidx[:n] = s_j.astype(np.int16)
    didx[:n] = d_loc.astype(np.int16)
    eid[:n] = ids_j
    # Within each gather chunk, order edges by src: the hu gather's HBM reads
    # become ascending per chunk (row locality) while dst stays chunk-local
    # sorted. Pure host permutation — the device program is unchanged.
    for c in range(nchunk):
        sl = slice(c * CHUNK_E, (c + 1) * CHUNK_E)
        perm = np.argsort(sidx[sl], kind="stable")
        sidx[sl] = sidx[sl][perm]
        didx[sl] = didx[sl][perm]
        eid[sl] = eid[sl][perm]
    return _wrap_idx(sidx, nchunk), _wrap_idx(didx, nchunk), eid


def kernel(ufeat, ifeat, src, dst):
    from concourse.bass_utils import run_bass_kernel_spmd

    ufeat_b = np.ascontiguousarray(np.asarray(ufeat, dtype=np.float32)).astype(BF16)
    ifeat_b = np.ascontiguousarray(np.asarray(ifeat, dtype=np.float32)).astype(BF16)
    src_f = np.asarray(src).ravel().astype(np.int64)
    dst_f = np.asarray(dst).ravel().astype(np.int64)
    assert src_f.shape == (E,) and dst_f.shape == (E,)

    cores = []
    for j in range(NCORES):
        lo, hi = j * ECORE, (j + 1) * ECORE
        d_j = dst_f[lo:hi]
        order = np.argsort(d_j, kind="stable")
        d_sorted = d_j[order]
        uniq, d_loc = np.unique(d_sorted, return_inverse=True)
        cores.append((src_f[lo:hi][order], d_loc, uniq, np.arange(lo, hi)[order]))

    vcap = max(len(u) for (_, _, u, _) in cores)
    nchunk = _cdiv(ECORE, CHUNK_E)

    key = (nchunk, vcap)
    if key not in _PROGRAM_CACHE:
        _PROGRAM_CACHE[key] = _build_program(nchunk, vcap)
    nc = _PROGRAM_CACHE[key]

    in_maps = []
    eids = []
    for j in range(NCORES):
        s_j, d_loc, uniq, ids_j = cores[j]
        vtab = np.zeros((vcap, D), BF16)
        vtab[: len(uniq)] = ifeat_b[uniq]
        sidx_w, didx_w, eid = _prep_core(s_j, d_loc, ids_j, nchunk)
        in_maps.append({"ufeat": ufeat_b, "vtab": vtab, "sidx": sidx_w, "didx": didx_w})
        eids.append(eid)

    res = run_bass_kernel_spmd(nc, in_maps, core_ids=list(range(NCORES)))

    out = np.empty((E, 1), np.float32)
    for j in range(NCORES):
        yj = res.results[j]["y"]          # [128, ntiles]; slot i -> y[i%128, i//128]
        vals = np.ascontiguousarray(yj.T).ravel()
        m = eids[j] >= 0
        out[eids[j][m], 0] = vals[m]
    return out


# revision 4
# speedup vs baseline: 1.5762x; 1.5762x over previous
"""Trainium2 Bass kernel for BiDecoder edge dot products.

out[e] = dot(ufeat[src[e]], ifeat[dst[e]])   for E=300000 edges, D=256.

Strategy (8 NeuronCores, SPMD):
  - Shard edges across the 8 cores (37500 each); replicate ufeat, and give
    each core a host-compacted ifeat table holding only its distinct dst
    rows (~26.4k < 32767, so one int16 gather base — no 32768-row split).
  - Tables are cast to bf16 on host (rel err ~1e-3 << 2e-2 gate): halves
    the gather traffic and doubles DVE throughput.
  - Per core, edges are sorted by dst (compacted hv rows are then read in
    ascending order) and, within each gather chunk, by src (hu reads
    ascend per chunk). Large 4096-row dma_gather calls amortize the
    per-call SWDGE fixed overhead on the Pool engine; 4 SWDGE queues
    rotate per call so generation overlaps transfers.
  - DVE affine_mul_reduce fuses multiply + row-sum (f32 accumulate); one
    final DMA writes all dots out. Host reorders to original edge order.
"""

import sys

for _p in ("/opt/trn_rl_repo",):
    if _p not in sys.path:
        sys.path.append(_p)

import numpy as np
import ml_dtypes

BF16 = ml_dtypes.bfloat16

P = 128
D = 256
E = 300000
NCORES = 8
ECORE = E // NCORES
N_GENE = 20000
N_CELL = 50000
C_TILE = 8               # tiles (of 128 edges) per gather chunk
CHUNK_E = C_TILE * P     # 1024 edges per dma_gather call (fits the 128-entry
                         # SWDGE ring: n/16+1 = 65 ring descs; 4096-row calls
                         # at 257 descs stall mid-generation — measured 7.9 vs
                         # 3.0 ns/row)
COLS = CHUNK_E // 16     # idx columns per chunk in the wrapped layout
NSLOT = 8                # buffer slots (chunk c uses slot c % NSLOT)

_PROGRAM_CACHE: dict = {}


def _cdiv(a, b):
    return -(-a // b)


def _wrap_idx(idx_i16: np.ndarray, nchunk: int) -> np.ndarray:
    """[nchunk*CHUNK_E] int16 -> [128, nchunk*COLS] dma_gather idx layout.

    Within each chunk, index i lives at partition i%16, column i//16; the
    16-partition block is replicated 8x down the 128 partitions.
    """
    w = idx_i16.reshape(nchunk, COLS, 16).transpose(2, 0, 1).reshape(16, nchunk * COLS)
    return np.ascontiguousarray(np.tile(w, (8, 1)))


def _build_program(nchunk: int, vcap: int, n_gene: int = N_GENE):
    import concourse.bacc as bacc
    import concourse.mybir as mybir
    from concourse.library_config import mlp

    ntiles = nchunk * C_TILE
    totcols = nchunk * COLS

    nc = bacc.Bacc("TRN2", debug=False, num_swdge_queues=4,
                   dynamic_dma_scratch_size=65536)
    ufeat = nc.dram_tensor("ufeat", [n_gene, D], mybir.dt.bfloat16, kind="ExternalInput")
    vtab = nc.dram_tensor("vtab", [vcap, D], mybir.dt.bfloat16, kind="ExternalInput")
    sidx = nc.dram_tensor("sidx", [P, totcols], mybir.dt.int16, kind="ExternalInput")
    didx = nc.dram_tensor("didx", [P, totcols], mybir.dt.int16, kind="ExternalInput")
    y = nc.dram_tensor("y", [P, ntiles], mybir.dt.float32, kind="ExternalOutput")

    with (
        nc.sbuf_tensor("hu", [P, NSLOT, C_TILE, D], mybir.dt.bfloat16) as hu,
        nc.sbuf_tensor("hv", [P, NSLOT, C_TILE, D], mybir.dt.bfloat16) as hv,
        nc.sbuf_tensor("sidx_sb", [P, totcols], mybir.dt.int16) as sidx_sb,
        nc.sbuf_tensor("didx_sb", [P, totcols], mybir.dt.int16) as didx_sb,
        nc.sbuf_tensor("osb", [P, ntiles], mybir.dt.float32) as osb,
        nc.semaphore("io") as io,
        nc.semaphore("cons") as cons,
        nc.semaphore("io2") as io2,
        nc.Block() as block,
        __import__("contextlib").ExitStack() as _stk,
    ):
        gu = [_stk.enter_context(nc.semaphore(f"gu{i}")) for i in range(NSLOT)]
        gv = [_stk.enter_context(nc.semaphore(f"gv{i}")) for i in range(NSLOT)]

        @block.gpsimd
        def _(gp):
            gp.load_library(mlp)
            gp.wait_ge(io, 32)
            for c in range(nchunk):
                s = c % NSLOT
                if c >= NSLOT:
                    gp.wait_ge(cons, c - NSLOT + 1)
                cols = slice(c * COLS, (c + 1) * COLS)
                gp.dma_gather(
                    hu[:, s], ufeat[:, :], sidx_sb[:, cols], CHUNK_E, CHUNK_E, D,
                    queue_num=(2 * c) % 4, single_packet=False,
                ).then_inc(gu[s], 16)
                gp.dma_gather(
                    hv[:, s], vtab[:, :], didx_sb[:, cols], CHUNK_E, CHUNK_E, D,
                    queue_num=(2 * c + 1) % 4, single_packet=False,
                ).then_inc(gv[s], 16)
            for s in range(NSLOT):
                cnt = (nchunk - s + NSLOT - 1) // NSLOT
                if cnt:
                    gp.wait_ge(gu[s], 16 * cnt)
                    gp.wait_ge(gv[s], 16 * cnt)

        @block.vector
        def _(v):
            for c in range(nchunk):
                s = c % NSLOT
                k = c // NSLOT + 1
                v.wait_ge(gu[s], 16 * k)
                v.wait_ge(gv[s], 16 * k)
                # Whole-chunk product then per-tile row sums: 2 DVE
                # instructions per chunk instead of C_TILE fused ops —
                # the custom-op fixed overhead dominated at [P,256].
                v.scalar_tensor_tensor(
                    out=hv[:, s],
                    in0=hu[:, s],
                    scalar=0.0,
                    in1=hv[:, s],
                    op0=mybir.AluOpType.bypass,
                    op1=mybir.AluOpType.mult,
                )
                v.tensor_reduce(
                    out=osb[:, c * C_TILE : (c + 1) * C_TILE],
                    in_=hv[:, s],
                    axis=mybir.AxisListType.X,
                    op=mybir.AluOpType.add,
                ).then_inc(cons, 1)

        @block.sync
        def _(sy):
            sy.dma_start(sidx_sb[:], sidx[:]).then_inc(io, 16)
            sy.dma_start(didx_sb[:], didx[:]).then_inc(io, 16)
            sy.wait_ge(cons, nchunk)
            sy.dma_start(y[:, :], osb[:, :]).then_inc(io2, 16)
            sy.wait_ge(io2, 16)

    nc.compile()
    return nc


def _prep_core(s_j, d_loc, ids_j, nchunk):
    """Build one core's slot arrays: wrapped int16 idx tensors + edge ids."""
    nslot_e = nchunk * CHUNK_E
    n = len(s_j)
    sidx = np.zeros(nslot_e, np.int16)
    didx = np.zeros(nslot_e, np.int16)
    eid = np.full(nslot_e, -1, np.int64)
    sidx[:n] = s_j.astype(np.int16)
    didx[:n] = d_loc.astype(np.int16)
    eid[:n] = ids_j
    # Within each gather chunk, order edges by src: the hu gather's HBM reads
    # become ascending per chunk (row locality) while dst stays chunk-local
    # sorted. Pure host permutation — the device program is unchanged.
    for c in range(nchunk):
        sl = slice(c * CHUNK_E, (c + 1) * CHUNK_E)
        perm = np.argsort(sidx[sl], kind="stable")
        sidx[sl] = sidx[sl][perm]
        didx[sl] = didx[sl][perm]
        eid[sl] = eid[sl][perm]
    return _wrap_idx(sidx, nchunk), _wrap_idx(didx, nchunk), eid


def kernel(ufeat, ifeat, src, dst):
    from concourse.bass_utils import run_bass_kernel_spmd

    ufeat_b = np.ascontiguousarray(np.asarray(ufeat, dtype=np.float32)).astype(BF16)
    ifeat_b = np.ascontiguousarray(np.asarray(ifeat, dtype=np.float32)).astype(BF16)
    src_f = np.asarray(src).ravel().astype(np.int64)
    dst_f = np.asarray(dst).ravel().astype(np.int64)
    assert src_f.shape == (E,) and dst_f.shape == (E,)

    cores = []
    for j in range(NCORES):
        lo, hi = j * ECORE, (j + 1) * ECORE
        d_j = dst_f[lo:hi]
        order = np.argsort(d_j, kind="stable")
        d_sorted = d_j[order]
        uniq, d_loc = np.unique(d_sorted, return_inverse=True)
        cores.append((src_f[lo:hi][order], d_loc, uniq, np.arange(lo, hi)[order]))

    vcap = max(len(u) for (_, _, u, _) in cores)
    nchunk = _cdiv(ECORE, CHUNK_E)

    key = (nchunk, vcap)
    if key not in _PROGRAM_CACHE:
        _PROGRAM_CACHE[key] = _build_program(nchunk, vcap)
    nc = _PROGRAM_CACHE[key]

    in_maps = []
    eids = []
    for j in range(NCORES):
        s_j, d_loc, uniq, ids_j = cores[j]
        vtab = np.zeros((vcap, D), BF16)
        vtab[: len(uniq)] = ifeat_b[uniq]
        sidx_w, didx_w, eid = _prep_core(s_j, d_loc, ids_j, nchunk)
        in_maps.append({"ufeat": ufeat_b, "vtab": vtab, "sidx": sidx_w, "didx": didx_w})
        eids.append(eid)

    res = run_bass_kernel_spmd(nc, in_maps, core_ids=list(range(NCORES)))

    out = np.empty((E, 1), np.float32)
    for j in range(NCORES):
        yj = res.results[j]["y"]          # [128, ntiles]; slot i -> y[i%128, i//128]
        vals = np.ascontiguousarray(yj.T).ravel()
        m = eids[j] >= 0
        out[eids[j][m], 0] = vals[m]
    return out
